# revision 84
# baseline (speedup 1.0000x reference)
"""Trainium2 Bass kernel for nn_Attention_86698209837214.

Multi-head attention: out = softmax(q k^T / 8) v @ W_out + b_out with
B=4, N=2048, DIM=1024, H=16, Dh=64.

Sharding: 8 cores = (batch b in 0..3) x (head-half hh in 0..1); each core
computes 8 heads of one batch. Host pre-transposes x[b], slices weights and
converts everything to bf16; host adds the two head-half partial outputs
plus b_out.

Device dataflow per core (bf16 operands, fp32 PSUM):
  1. v = x @ Wv in natural [n, c] layout, augmented with a ones column per
     head slot (row 64 of the attn@v accumulator = softmax denominator).
  2. qT, kT = (x @ Wq/Wk)^T in [c, n] layout (lhsT = W tiles).
  3. Attention, ic (i-chunk of 512) outer, hp (head pair) inner:
     dots^T per head via K=64 row-packed matmuls (tile_position r0=64*s, the
     two head streams run on disjoint PE row groups and overlap),
     exp split across two engines: ScalarE table exp (bf16 out) and DVE
     Schraudolph exp (tensor_scalar mult+add -> round-to-int16 == bf16 bit
     pattern; ~2% weight noise that cancels in softmax normalization),
     attn@v accumulated over j in PSUM with M=65 (65th row = denominator);
     attn@v lags dots by one jp so the PE never waits on the current exp.
     Epilogue: av -> SBUF copies (frees PSUM banks), denominator row to
     partition 0 via DMA hop, fast reciprocal, gpsimd partition_broadcast,
     gpsimd muls into aT (normalize); s=1 half reaches aT partitions 64:127
     via SBUF-to-SBUF DMA.
  4. Out-projection per ic accumulates all 4 head pairs in PSUM (K=128 x 4)
     -> single bf16 result DMA'd out; host adds the 2 cores + b_out.
"""

import sys

for _p in ("/opt/trn_rl_repo",):
    if _p not in sys.path:
        sys.path.append(_p)

from contextlib import ExitStack

import ml_dtypes
import numpy as np

import concourse.bass as bass  # noqa: F401
import concourse.tile as tile
from concourse import bacc, mybir
from concourse.bass_utils import run_bass_kernel_spmd

F32 = mybir.dt.float32
F32R = mybir.dt.float32r
BF16 = mybir.dt.bfloat16
I16 = mybir.dt.int16
AF = mybir.ActivationFunctionType
ALU = mybir.AluOpType

P = 128
NSEQ = 2048  # sequence length per batch
D = 1024  # model dim
CH = 512  # per-core head-dim width (8 heads x 64)
DH = 64
NPAIR = 4  # head pairs per core (c-tiles of 128)
NDT = D // P  # 8 d-tiles
NNT = NSEQ // P  # 16 n-tiles
NNC = NSEQ // 512  # 4 i-chunks
NJP = NNT // 2  # 8 jp steps per chunk
SCALE = 0.125  # DIM_HEAD ** -0.5

# Schraudolph exp in bf16-bit-pattern domain (scale folded in):
#   int16 bits = round(dots * SCALE * 2^7/ln2 + (127*128 - 486411/2^16))
A_SCH = SCALE * 184.6650390625
B_SCH = 16248.576

# (jp, s) pairs whose exp runs on DVE (Schraudolph); rest on ScalarE.
DVE_EXP = {(jp, 1) for jp in range(NSEQ // 256)}


def build_program():
    nc = bacc.Bacc("TRN2", target_bir_lowering=False, debug=False)

    # host-shuffled layouts: partition-major so each partition's DMA run is
    # long and contiguous (DMA engines are packet-rate-bound on short runs)
    xt = nc.dram_tensor("xt", [P, NDT, NSEQ], BF16, kind="ExternalInput")
    wqkv = nc.dram_tensor("wqkv", [P, 3, NDT, CH], BF16, kind="ExternalInput")
    wout = nc.dram_tensor("wout", [P, NPAIR, D], BF16, kind="ExternalInput")
    ones_in = nc.dram_tensor("ones", [P, 1], BF16, kind="ExternalInput")
    out = nc.dram_tensor("out", [NSEQ, D], BF16, kind="ExternalOutput")

    out_t = out.ap().rearrange("(nt p) e -> nt p e", p=P)  # [16, 128, 1024]

    copy_flip = [0]

    with tile.TileContext(nc) as tc, ExitStack() as ctx:
        # ---- persistent pools ----
        p_qk = ctx.enter_context(tc.tile_pool(name="p_qk", bufs=1))  # 32 KB/p
        p_v = ctx.enter_context(tc.tile_pool(name="p_v", bufs=1))  # ~16 KB/p
        p_small = ctx.enter_context(tc.tile_pool(name="p_small", bufs=1))
        # PSUM: dots 2x[128,1024] (4 banks) + av 2x[65,512] (2) + out 2x (2)
        ps_mm = ctx.enter_context(tc.tile_pool(name="ps_mm", bufs=2, space="PSUM"))
        ps_av = ctx.enter_context(tc.tile_pool(name="ps_av", bufs=2, space="PSUM"))
        ps_out = ctx.enter_context(tc.tile_pool(name="ps_out", bufs=2, space="PSUM"))
        # attention-phase pools (created before the transient phase-A pools so
        # pool release order stays LIFO)
        p_exp = ctx.enter_context(tc.tile_pool(name="p_exp", bufs=12))  # 24 KB/p
        p_aT = ctx.enter_context(tc.tile_pool(name="p_aT", bufs=16))  # 16 KB/p
        p_wout = ctx.enter_context(tc.tile_pool(name="p_wout", bufs=1))  # 8 KB/p
        p_avsb = ctx.enter_context(tc.tile_pool(name="p_avsb", bufs=3))  # 6 KB/p
        p_den = ctx.enter_context(tc.tile_pool(name="p_den", bufs=2))
        p_recip = ctx.enter_context(tc.tile_pool(name="p_recip", bufs=2))
        p_bcast = ctx.enter_context(tc.tile_pool(name="p_bcast", bufs=3))
        p_tmp = ctx.enter_context(tc.tile_pool(name="p_tmp", bufs=2))
        p_ostage = ctx.enter_context(tc.tile_pool(name="p_ostage", bufs=4))

        ones = p_small.tile([P, 1], BF16, tag="ones")
        nc.sync.dma_start(out=ones, in_=ones_in.ap())
        # dummy exp: pulls the ~2.7us ACT_TABLE_LOAD into the initial DMA wait
        warm = p_small.tile([P, 1], F32, tag="warm")
        nc.scalar.activation(out=warm, in_=ones, func=AF.Exp, scale=1.0)
        # ones row for the PE-side reciprocal broadcast (K=1 matmul)
        ones_row = p_small.tile([1, DH], BF16, tag="ones_row")
        nc.vector.memset(ones_row, 1.0)

        def stage_copy(dst, src):
            # alternate PSUM->SBUF staging copies between DVE and ScalarE
            copy_flip[0] ^= 1
            if copy_flip[0]:
                nc.vector.tensor_copy(dst, src)
            else:
                nc.scalar.copy(dst, src)

        # ---- phase A: load xt, wv, wk; compute v_aug ----
        st_xt = ExitStack()
        p_xt = st_xt.enter_context(tc.tile_pool(name="p_xt", bufs=1))  # 32 KB/p
        st_wk = ExitStack()
        p_wk = st_wk.enter_context(tc.tile_pool(name="p_wk", bufs=1))
        st_wv = ExitStack()
        p_wv = st_wv.enter_context(tc.tile_pool(name="p_wv", bufs=1))

        # input DMAs: per-dt slices of xt and wv round-robin across all three
        # DMA queues, so the first tiles land ~6us in and the dt-outer v-proj
        # below starts streaming long before the full load completes
        wv_sb = p_wv.tile([P, NDT, CH], BF16, tag="wv")
        xt_sb = p_xt.tile([P, NDT, NSEQ], BF16, tag="xt")
        dma_q = [nc.scalar, nc.sync, nc.gpsimd]
        for dt_i in range(NDT):
            q = dma_q[dt_i % 3]
            q.dma_start(out=wv_sb[:, dt_i], in_=wqkv.ap()[:, 2, dt_i])
            # column-quartered: each v-proj matmul needs only a 128-col slice,
            # so 128KB arrival granularity keeps PE wait gaps well under the
            # ~3.4us HAM re-throttle window on slow-DMA cores
            for cq in range(4):
                q.dma_start(
                    out=xt_sb[:, dt_i, cq * 512 : (cq + 1) * 512],
                    in_=xt.ap()[:, dt_i, cq * 512 : (cq + 1) * 512],
                )
        wk_sb = p_wk.tile([P, NDT, CH], BF16, tag="wk")
        nc.scalar.dma_start(out=wk_sb, in_=wqkv.ap()[:, 1])
        xt_tiles = [xt_sb[:, dt_i] for dt_i in range(NDT)]
        wv_tiles = [wv_sb[:, dt_i] for dt_i in range(NDT)]
        wk_tiles = [wk_sb[:, dt_i] for dt_i in range(NDT)]

        # v_aug: per head-slot sg, 65 cols = [v_sg (64) | ones (1)]
        v_tiles = []
        for nt in range(NNT):
            dst = p_v.tile([P, 8 * 65], BF16, tag=f"v{nt}")
            ones_dst = dst.rearrange("p (h c) -> p h c", c=65)[:, :, 64:65]
            nc.gpsimd.memset(ones_dst, 1.0)
            v_tiles.append(dst)
        # dt-outer over 8 parallel PSUM accumulators (all 8 banks are free at
        # startup): each dt's matmuls issue as soon as that dt slice arrives
        vacc = []
        for nm in ("vA", "vB"):
            t = ps_mm.tile([P, 1024], F32, tag="mm", name=nm)
            vacc += [t[:, 0:512], t[:, 512:1024]]
        for s in range(2):
            t = ps_av.tile([P, 512], F32, tag="av", name=f"vC{s}")
            vacc.append(t)
        for s in range(2):
            t = ps_out.tile([P, 512], F32, tag="o", name=f"vD{s}")
            vacc.append(t)
        for half in range(2):
            for dt_i in range(NDT):
                for k in range(8):
                    nt = half * 8 + k
                    nc.tensor.matmul(
                        vacc[k],
                        xt_tiles[dt_i][:, nt * P : (nt + 1) * P],
                        wv_tiles[dt_i],
                        start=(dt_i == 0),
                        stop=(dt_i == NDT - 1),
                    )
            for k in range(8):
                dst = v_tiles[half * 8 + k]
                v_dst = dst.rearrange("p (h c) -> p h c", c=65)[:, :, 0:DH]
                stage_copy(v_dst, vacc[k].rearrange("p (h c) -> p h c", c=DH))
        st_wv.close()

        # ---- phase B: kT c-tiles, then qT c-tiles interleaved with attention
        st_wq = ExitStack()
        p_wq = st_wq.enter_context(tc.tile_pool(name="p_wq", bufs=1))
        wq_sb = p_wq.tile([P, NDT, CH], BF16, tag="wq")
        nc.scalar.dma_start(out=wq_sb, in_=wqkv.ap()[:, 0])
        wq_tiles = [wq_sb[:, dt_i] for dt_i in range(NDT)]

        kT_tiles = []
        qT_tiles = []

        def emit_qk_tile(which, w_tiles, ct):
            dst = p_qk.tile([P, NSEQ], BF16, tag=f"{which}T{ct}", name=f"{which}T{ct}")
            woff = ct * P
            for nch in range(NNC):
                acc = ps_mm.tile([P, 512], F32, tag="mm", name="acc")
                for dt_i in range(NDT):
                    nc.tensor.matmul(
                        acc,
                        w_tiles[dt_i][:, woff : woff + P],
                        xt_tiles[dt_i][:, nch * 512 : (nch + 1) * 512],
                        start=(dt_i == 0),
                        stop=(dt_i == NDT - 1),
                    )
                stage_copy(dst[:, nch * 512 : (nch + 1) * 512], acc)
            (kT_tiles if which == "k" else qT_tiles).append(dst)

        for ct in range(NPAIR):
            emit_qk_tile("k", wk_tiles, ct)
        emit_qk_tile("q", wq_tiles, 0)

        wout_sb = p_wout.tile([P, NPAIR, D], BF16, tag="wout")
        nc.gpsimd.dma_start(out=wout_sb, in_=wout.ap())
        wout_tiles = [wout_sb[:, ct] for ct in range(NPAIR)]

        # ---- attention: flat software pipeline across chunk boundaries ----
        # Per jp step of the CURRENT chunk: dots -> av of the step one behind
        # (possibly the previous chunk's tail) -> maybe one out-proj group ->
        # exp for this step. The PE's in-order queue therefore never waits on
        # an epilogue chain: epilogues and out-projections overlap the next
        # chunk's dots/av stream (>3.4us PE-idle gaps re-throttle HAM to
        # 1.2GHz, which is what made the naive ordering slow).
        aT_by_ic = {}  # ic -> [aT tile per hp]

        class Cctx:
            def __init__(self, ic, hp):
                self.ic, self.hp, self.i0 = ic, hp, ic * 512
                self.av_ps = [
                    ps_av.tile([65, 512], F32, tag="av", name=f"av{s}")
                    for s in range(2)
                ]
                self.aT = p_aT.tile([P, 512], BF16, tag="aT", name=f"aT{ic}_{hp}")
                aT_by_ic.setdefault(ic, []).append(self.aT)

        def emit_dots(c, jp):
            tiles = []
            for s in range(2):
                tiles.append(ps_mm.tile([P, 1024], F32, tag="mm", name="dots"))
            for half in range(2):
                for s in range(2):
                    r0 = s * DH
                    jtx = 2 * jp + half
                    nc.tensor.matmul(
                        tiles[s][:, half * 512 : (half + 1) * 512],
                        kT_tiles[c.hp][r0 : r0 + DH, jtx * P : (jtx + 1) * P],
                        qT_tiles[c.hp][r0 : r0 + DH, c.i0 : c.i0 + 512],
                        start=True,
                        stop=True,
                        tile_position=(r0, 0),
                    )
            return tiles

        def emit_exp(c, jp, dots_tiles):
            exp_tiles = []
            for s in range(2):
                e = p_exp.tile([P, 1024], BF16, tag="exp")
                if (jp, s) in DVE_EXP:
                    nc.vector.tensor_scalar(
                        out=e.bitcast(I16),
                        in0=dots_tiles[s],
                        scalar1=A_SCH,
                        scalar2=B_SCH,
                        op0=ALU.mult,
                        op1=ALU.add,
                    )
                else:
                    nc.scalar.activation(
                        out=e, in_=dots_tiles[s], func=AF.Exp, scale=SCALE
                    )
                exp_tiles.append(e)
            return exp_tiles

        def emit_av(c, jp, exp_pair):
            for s in range(2):
                sg = c.hp * 2 + s
                for half in range(2):
                    jtx = 2 * jp + half
                    nc.tensor.matmul(
                        c.av_ps[s],
                        v_tiles[jtx][:, sg * 65 : sg * 65 + 65],
                        exp_pair[s][:, half * 512 : (half + 1) * 512],
                        start=(jp == 0 and half == 0),
                        stop=(jp == NJP - 1 and half == 1),
                    )

        # Epilogue pipeline. The gpsimd must do NO compute here: its
        # partition_broadcast / tensor ops are ext-isa kernels that evict the
        # SWDGE DGE ucode from Q7 IRAM, and every swap costs a ~6us invisible
        # IRAM reload that was gating the PE once per chunk. Broadcast is a
        # K=1 ones-matmul on the PE instead; normalize-muls run on the DVE.
        def emit_epilogue_a(c, e):
            # evacuate av PSUM, hop denominator row to partition 0 (scalar
            # HWDGE queue: tiny, and ordered after the s=1 copy it needs)
            av_sb = []
            for s in range(2):
                t = p_avsb.tile([65, 512], F32, tag="av_sb", name=f"avsb{s}")
                if s == 0:
                    nc.vector.tensor_copy(t, c.av_ps[s])
                else:
                    nc.scalar.copy(t, c.av_ps[s])
                av_sb.append(t)
            e["av_sb"] = av_sb
            den_sb = p_den.tile([1, 1024], F32, tag="den_sb")
            for s in range(2):
                nc.scalar.dma_start(
                    out=den_sb[:, s * 512 : (s + 1) * 512], in_=av_sb[s][64:65, :]
                )
            e["den_sb"] = den_sb

        def emit_epilogue_b1(c, e):
            # two steps later: den hop has landed, recip doesn't block DVE.
            # SWDGE casting DMA converts f32 -> bf16 for the fp32r-averse
            # broadcast matmul (verifier rejects bitcast-f32r matmul inputs).
            recip = p_recip.tile([1, 1024], F32, tag="recip")
            nc.vector.reciprocal_approx_fast(out=recip, in_=e["den_sb"])
            recip_bf = p_recip.tile([1, 1024], BF16, tag="recip_bf")
            nc.gpsimd.dma_start(out=recip_bf, in_=recip)
            e["recip"] = recip_bf

        def emit_epilogue_b2(c, e):
            # PE broadcast: bc[d, i] = ones[d] * recip[i], K=1 matmul
            bcs = []
            for s in range(2):
                bc = ps_out.tile([DH, 512], F32, tag="o", name=f"bc{s}")
                nc.tensor.matmul(
                    bc,
                    ones_row,
                    e["recip"][:, s * 512 : (s + 1) * 512],
                    start=True,
                    stop=True,
                )
                bcs.append(bc)
            e["bc"] = bcs

        def emit_epilogue_b3(c, e):
            # normalize on DVE; s=1 reaches aT partitions 64:127 via an
            # SBUF-to-SBUF DMA hop (engines can't cross partitions)
            av_sb, bcs = e["av_sb"], e["bc"]
            nc.vector.tensor_mul(c.aT[0:DH, :], av_sb[0][0:DH, :], bcs[0])
            tmp = p_tmp.tile([DH, 512], BF16, tag="tmp")
            nc.vector.tensor_mul(tmp, av_sb[1][0:DH, :], bcs[1])
            nc.gpsimd.dma_start(out=c.aT[DH:P, :], in_=tmp)

        o_sb_by_nt = {}
        o_ps_cur = [None]

        def emit_outproj_mm(ic, lnt, ec, hp):
            # one accumulating matmul per pipeline step: each waits only on
            # its own hp's aT, so the PE never stalls on a full epilogue chain
            nt = 4 * ic + lnt
            if ec == 0 and hp == 0:
                o_sb_by_nt[nt] = p_ostage.tile(
                    [P, D], BF16, tag="o_sb", name=f"o_sb{nt}"
                )
            if hp == 0:
                o_ps_cur[0] = ps_out.tile([P, 512], F32, tag="o", name=f"o{nt}_{ec}")
            o_ps = o_ps_cur[0]
            nc.tensor.matmul(
                o_ps,
                aT_by_ic[ic][hp][:, lnt * P : (lnt + 1) * P],
                wout_tiles[hp][:, ec * 512 : (ec + 1) * 512],
                start=(hp == 0),
                stop=(hp == NPAIR - 1),
            )
            if hp == NPAIR - 1:
                stage_copy(o_sb_by_nt[nt][:, ec * 512 : (ec + 1) * 512], o_ps)
                if ec == 1:
                    nc.sync.dma_start(out=out_t[nt], in_=o_sb_by_nt.pop(nt))

        from collections import deque

        pend_av = deque()  # (cctx, jp, exp_tiles); av lags dots by AV_LAG steps
        AV_LAG = 3  # exp gets three pipeline steps of slack before av needs it
        pend_tasks = []  # [countdown, fn] epilogue stages
        pend_out = deque()  # (ic, lnt, ec, hp) mms
        out_gate = 0  # steps to wait before pumping out-proj (aT DMA latency)

        def pump(drain=False):
            nonlocal out_gate
            if pend_av and (len(pend_av) >= AV_LAG or drain):
                pc, pjp, pexp = pend_av.popleft()
                emit_av(pc, pjp, pexp)
                if pjp == NJP - 1:
                    e = {}
                    emit_epilogue_a(pc, e)
                    for delay, fn in (
                        (2, emit_epilogue_b1),
                        (4, emit_epilogue_b2),
                        (5, emit_epilogue_b3),
                    ):
                        pend_tasks.append([delay, fn, pc, e])
                    if pc.hp == NPAIR - 1 and pc.ic < NNC - 1:
                        pend_out.extend(
                            (pc.ic, lnt, ec, hp)
                            for lnt in range(4)
                            for ec in range(2)
                            for hp in range(NPAIR)
                        )
                        out_gate = 12
            for t in list(pend_tasks):
                t[0] -= 1
                if t[0] <= 0:
                    t[1](t[2], t[3])
                    pend_tasks.remove(t)
            if out_gate <= 0:
                for _ in range(2 if len(pend_out) > 8 else 1):
                    if pend_out:
                        emit_outproj_mm(*pend_out.popleft())
            out_gate -= 1

        chunks = [(ic, hp) for ic in range(NNC) for hp in range(NPAIR)]
        for ci, (ic, hp) in enumerate(chunks):
            c = Cctx(ic, hp)
            for jp in range(NJP):
                dots_tiles = emit_dots(c, jp)
                pump()
                exp_tiles = emit_exp(c, jp, dots_tiles)
                pend_av.append((c, jp, exp_tiles))
            if ic == 0 and hp + 1 < NPAIR:
                # remaining qT tiles ride between the first chunks' streams
                emit_qk_tile("q", wq_tiles, hp + 1)
        # drain av tail + epilogues
        while pend_av or pend_tasks or pend_out:
            pump(drain=True)

        # final-ic out-projection: the last chunk's epilogue chain (~6us)
        # would stall every group at its hp3 matmul with only 2 PSUM slots.
        # The dots/av banks are free now — borrow them so 6 groups run their
        # ready hp0-2 matmuls as cover while that epilogue completes.
        ICL = NNC - 1
        fgroups = [(lnt, ec) for lnt in range(4) for ec in range(2)]
        o_ps_f = []
        for gi in range(4):
            pool, ptag = (ps_out, "o") if gi < 2 else (ps_av, "av")
            t = pool.tile([P, 512], F32, tag=ptag, name=f"fo{gi}")
            o_ps_f.append(t)
        for nm in ("fmA", "fmB"):
            t = pool = ps_mm.tile([P, 1024], F32, tag="mm", name=nm)
            o_ps_f += [t[:, 0:512], t[:, 512:1024]]
        for gi in range(8):
            lnt, ec = fgroups[gi]
            if ec == 0:
                o_sb_by_nt[4 * ICL + lnt] = p_ostage.tile(
                    [P, D], BF16, tag="o_sb", name=f"fsb{lnt}"
                )
        for hp in range(NPAIR - 1):
            for gi in range(8):
                lnt, ec = fgroups[gi]
                nc.tensor.matmul(
                    o_ps_f[gi],
                    aT_by_ic[ICL][hp][:, lnt * P : (lnt + 1) * P],
                    wout_tiles[hp][:, ec * 512 : (ec + 1) * 512],
                    start=(hp == 0),
                    stop=False,
                )
        for gi in range(8):
            lnt, ec = fgroups[gi]
            nc.tensor.matmul(
                o_ps_f[gi],
                aT_by_ic[ICL][NPAIR - 1][:, lnt * P : (lnt + 1) * P],
                wout_tiles[NPAIR - 1][:, ec * 512 : (ec + 1) * 512],
                start=False,
                stop=True,
            )
            stage_copy(o_sb_by_nt[4 * ICL + lnt][:, ec * 512 : (ec + 1) * 512], o_ps_f[gi])
            if ec == 1:
                nc.sync.dma_start(
                    out=out_t[4 * ICL + lnt], in_=o_sb_by_nt.pop(4 * ICL + lnt)
                )
        st_wq.close()
        st_wk.close()
        st_xt.close()

    nc.compile()
    return nc


_NC = None


def _get_program():
    global _NC
    if _NC is None:
        _NC = build_program()
    return _NC


INNER = 1024
BF = ml_dtypes.bfloat16


def kernel(x, W_qkv, W_out, b_out):
    x = np.asarray(x, dtype=np.float32)
    W_qkv = np.asarray(W_qkv, dtype=np.float32)
    W_out = np.asarray(W_out, dtype=np.float32)
    b_out = np.asarray(b_out, dtype=np.float32)
    B = x.shape[0]

    nc = _get_program()
    in_maps = []
    for b in range(B):
        for hh in range(2):
            cs = hh * CH
            wq = W_qkv[:, cs : cs + CH]
            wk = W_qkv[:, INNER + cs : INNER + cs + CH]
            wv = W_qkv[:, 2 * INNER + cs : 2 * INNER + cs + CH]
            # partition-major shuffles so per-partition DMA runs are contiguous
            xt_sh = np.ascontiguousarray(
                x[b].T.reshape(NDT, P, NSEQ).transpose(1, 0, 2)
            ).astype(BF)
            wqkv_sh = np.ascontiguousarray(
                np.stack([wq, wk, wv])  # [3, 1024, 512]
                .reshape(3, NDT, P, CH)
                .transpose(2, 0, 1, 3)
            ).astype(BF)
            wout_sh = np.ascontiguousarray(
                W_out[cs : cs + CH, :].reshape(NPAIR, P, D).transpose(1, 0, 2)
            ).astype(BF)
            in_maps.append(
                {
                    "xt": xt_sh,
                    "wqkv": wqkv_sh,
                    "wout": wout_sh,
                    "ones": np.ones((P, 1), dtype=BF),
                }
            )
    res = run_bass_kernel_spmd(nc, in_maps, core_ids=list(range(8)))
    out = np.empty((B, NSEQ, D), dtype=np.float32)
    for b in range(B):
        out[b] = (
            res.results[2 * b]["out"].astype(np.float32)
            + res.results[2 * b + 1]["out"].astype(np.float32)
            + b_out
        )
    return out


# revision 85
# speedup vs baseline: 1.0050x; 1.0050x over previous
"""Trainium2 Bass kernel for nn_Attention_86698209837214.

Multi-head attention: out = softmax(q k^T / 8) v @ W_out + b_out with
B=4, N=2048, DIM=1024, H=16, Dh=64.

Sharding: 8 cores = (batch b in 0..3) x (head-half hh in 0..1); each core
computes 8 heads of one batch. Host pre-transposes x[b], slices weights and
converts everything to bf16; host adds the two head-half partial outputs
plus b_out.

Device dataflow per core (bf16 operands, fp32 PSUM):
  1. v = x @ Wv in natural [n, c] layout, augmented with a ones column per
     head slot (row 64 of the attn@v accumulator = softmax denominator).
  2. qT, kT = (x @ Wq/Wk)^T in [c, n] layout (lhsT = W tiles).
  3. Attention, ic (i-chunk of 512) outer, hp (head pair) inner:
     dots^T per head via K=64 row-packed matmuls (tile_position r0=64*s, the
     two head streams run on disjoint PE row groups and overlap),
     exp split across two engines: ScalarE table exp (bf16 out) and DVE
     Schraudolph exp (tensor_scalar mult+add -> round-to-int16 == bf16 bit
     pattern; ~2% weight noise that cancels in softmax normalization),
     attn@v accumulated over j in PSUM with M=65 (65th row = denominator);
     attn@v lags dots by one jp so the PE never waits on the current exp.
     Epilogue: av -> SBUF copies (frees PSUM banks), denominator row to
     partition 0 via DMA hop, fast reciprocal, gpsimd partition_broadcast,
     gpsimd muls into aT (normalize); s=1 half reaches aT partitions 64:127
     via SBUF-to-SBUF DMA.
  4. Out-projection per ic accumulates all 4 head pairs in PSUM (K=128 x 4)
     -> single bf16 result DMA'd out; host adds the 2 cores + b_out.
"""

import sys

for _p in ("/opt/trn_rl_repo",):
    if _p not in sys.path:
        sys.path.append(_p)

from contextlib import ExitStack

import ml_dtypes
import numpy as np

import concourse.bass as bass  # noqa: F401
import concourse.tile as tile
from concourse import bacc, mybir
from concourse.bass_utils import run_bass_kernel_spmd

F32 = mybir.dt.float32
F32R = mybir.dt.float32r
BF16 = mybir.dt.bfloat16
I16 = mybir.dt.int16
AF = mybir.ActivationFunctionType
ALU = mybir.AluOpType

P = 128
NSEQ = 2048  # sequence length per batch
D = 1024  # model dim
CH = 512  # per-core head-dim width (8 heads x 64)
DH = 64
NPAIR = 4  # head pairs per core (c-tiles of 128)
NDT = D // P  # 8 d-tiles
NNT = NSEQ // P  # 16 n-tiles
NNC = NSEQ // 512  # 4 i-chunks
NJP = NNT // 2  # 8 jp steps per chunk
SCALE = 0.125  # DIM_HEAD ** -0.5

# Schraudolph exp in bf16-bit-pattern domain (scale folded in):
#   int16 bits = round(dots * SCALE * 2^7/ln2 + (127*128 - 486411/2^16))
A_SCH = SCALE * 184.6650390625
B_SCH = 16248.576

# (jp, s) pairs whose exp runs on DVE (Schraudolph); rest on ScalarE.
DVE_EXP = {(jp, 1) for jp in range(NSEQ // 256)}


def build_program():
    nc = bacc.Bacc("TRN2", target_bir_lowering=False, debug=False)

    # host-shuffled layouts: partition-major so each partition's DMA run is
    # long and contiguous (DMA engines are packet-rate-bound on short runs)
    xt = nc.dram_tensor("xt", [P, NDT, NSEQ], BF16, kind="ExternalInput")
    wqkv = nc.dram_tensor("wqkv", [P, 3, NDT, CH], BF16, kind="ExternalInput")
    wout = nc.dram_tensor("wout", [P, NPAIR, D], BF16, kind="ExternalInput")
    ones_in = nc.dram_tensor("ones", [P, 1], BF16, kind="ExternalInput")
    out = nc.dram_tensor("out", [NSEQ, D], BF16, kind="ExternalOutput")

    out_t = out.ap().rearrange("(nt p) e -> nt p e", p=P)  # [16, 128, 1024]

    copy_flip = [0]

    with tile.TileContext(nc) as tc, ExitStack() as ctx:
        # ---- persistent pools ----
        p_qk = ctx.enter_context(tc.tile_pool(name="p_qk", bufs=1))  # 32 KB/p
        p_v = ctx.enter_context(tc.tile_pool(name="p_v", bufs=1))  # ~16 KB/p
        p_small = ctx.enter_context(tc.tile_pool(name="p_small", bufs=1))
        # PSUM: dots 2x[128,1024] (4 banks) + av 2x[65,512] (2) + out 2x (2)
        ps_mm = ctx.enter_context(tc.tile_pool(name="ps_mm", bufs=2, space="PSUM"))
        ps_av = ctx.enter_context(tc.tile_pool(name="ps_av", bufs=2, space="PSUM"))
        ps_out = ctx.enter_context(tc.tile_pool(name="ps_out", bufs=2, space="PSUM"))
        # attention-phase pools (created before the transient phase-A pools so
        # pool release order stays LIFO)
        p_exp = ctx.enter_context(tc.tile_pool(name="p_exp", bufs=12))  # 24 KB/p
        p_aT = ctx.enter_context(tc.tile_pool(name="p_aT", bufs=16))  # 16 KB/p
        p_wout = ctx.enter_context(tc.tile_pool(name="p_wout", bufs=1))  # 8 KB/p
        p_avsb = ctx.enter_context(tc.tile_pool(name="p_avsb", bufs=3))  # 6 KB/p
        p_den = ctx.enter_context(tc.tile_pool(name="p_den", bufs=2))
        p_recip = ctx.enter_context(tc.tile_pool(name="p_recip", bufs=2))
        p_bcast = ctx.enter_context(tc.tile_pool(name="p_bcast", bufs=3))
        p_tmp = ctx.enter_context(tc.tile_pool(name="p_tmp", bufs=2))
        p_ostage = ctx.enter_context(tc.tile_pool(name="p_ostage", bufs=3))

        ones = p_small.tile([P, 1], BF16, tag="ones")
        nc.sync.dma_start(out=ones, in_=ones_in.ap())
        # dummy exp: pulls the ~2.7us ACT_TABLE_LOAD into the initial DMA wait
        warm = p_small.tile([P, 1], F32, tag="warm")
        nc.scalar.activation(out=warm, in_=ones, func=AF.Exp, scale=1.0)
        # ones row for the PE-side reciprocal broadcast (K=1 matmul)
        ones_row = p_small.tile([1, DH], BF16, tag="ones_row")
        nc.vector.memset(ones_row, 1.0)

        def stage_copy(dst, src):
            # alternate PSUM->SBUF staging copies between DVE and ScalarE
            copy_flip[0] ^= 1
            if copy_flip[0]:
                nc.vector.tensor_copy(dst, src)
            else:
                nc.scalar.copy(dst, src)

        # ---- phase A: load xt, wv, wk; compute v_aug ----
        st_xt = ExitStack()
        p_xt = st_xt.enter_context(tc.tile_pool(name="p_xt", bufs=1))  # 32 KB/p
        st_wk = ExitStack()
        p_wk = st_wk.enter_context(tc.tile_pool(name="p_wk", bufs=1))
        st_wv = ExitStack()
        p_wv = st_wv.enter_context(tc.tile_pool(name="p_wv", bufs=1))

        # input DMAs: per-dt slices of xt and wv round-robin across all three
        # DMA queues, so the first tiles land ~6us in and the dt-outer v-proj
        # below starts streaming long before the full load completes
        wv_sb = p_wv.tile([P, NDT, CH], BF16, tag="wv")
        xt_sb = p_xt.tile([P, NDT, NSEQ], BF16, tag="xt")
        dma_q = [nc.scalar, nc.sync, nc.gpsimd]
        for dt_i in range(NDT):
            q = dma_q[dt_i % 3]
            q.dma_start(out=wv_sb[:, dt_i], in_=wqkv.ap()[:, 2, dt_i])
            # column-quartered: each v-proj matmul needs only a 128-col slice,
            # so 128KB arrival granularity keeps PE wait gaps well under the
            # ~3.4us HAM re-throttle window on slow-DMA cores
            for cq in range(4):
                q.dma_start(
                    out=xt_sb[:, dt_i, cq * 512 : (cq + 1) * 512],
                    in_=xt.ap()[:, dt_i, cq * 512 : (cq + 1) * 512],
                )
        wk_sb = p_wk.tile([P, NDT, CH], BF16, tag="wk")
        nc.scalar.dma_start(out=wk_sb, in_=wqkv.ap()[:, 1])
        xt_tiles = [xt_sb[:, dt_i] for dt_i in range(NDT)]
        wv_tiles = [wv_sb[:, dt_i] for dt_i in range(NDT)]
        wk_tiles = [wk_sb[:, dt_i] for dt_i in range(NDT)]

        # v_aug: per head-slot sg, 65 cols = [v_sg (64) | ones (1)]
        v_tiles = []
        for nt in range(NNT):
            dst = p_v.tile([P, 8 * 65], BF16, tag=f"v{nt}")
            ones_dst = dst.rearrange("p (h c) -> p h c", c=65)[:, :, 64:65]
            nc.gpsimd.memset(ones_dst, 1.0)
            v_tiles.append(dst)
        # dt-outer over 8 parallel PSUM accumulators (all 8 banks are free at
        # startup): each dt's matmuls issue as soon as that dt slice arrives
        vacc = []
        for nm in ("vA", "vB"):
            t = ps_mm.tile([P, 1024], F32, tag="mm", name=nm)
            vacc += [t[:, 0:512], t[:, 512:1024]]
        for s in range(2):
            t = ps_av.tile([P, 512], F32, tag="av", name=f"vC{s}")
            vacc.append(t)
        for s in range(2):
            t = ps_out.tile([P, 512], F32, tag="o", name=f"vD{s}")
            vacc.append(t)
        for half in range(2):
            for dt_i in range(NDT):
                for k in range(8):
                    nt = half * 8 + k
                    nc.tensor.matmul(
                        vacc[k],
                        xt_tiles[dt_i][:, nt * P : (nt + 1) * P],
                        wv_tiles[dt_i],
                        start=(dt_i == 0),
                        stop=(dt_i == NDT - 1),
                    )
            for k in range(8):
                dst = v_tiles[half * 8 + k]
                v_dst = dst.rearrange("p (h c) -> p h c", c=65)[:, :, 0:DH]
                stage_copy(v_dst, vacc[k].rearrange("p (h c) -> p h c", c=DH))
        st_wv.close()

        # ---- phase B: kT c-tiles, then qT c-tiles interleaved with attention
        st_wq = ExitStack()
        p_wq = st_wq.enter_context(tc.tile_pool(name="p_wq", bufs=1))
        wq_sb = p_wq.tile([P, NDT, CH], BF16, tag="wq")
        nc.scalar.dma_start(out=wq_sb, in_=wqkv.ap()[:, 0])
        wq_tiles = [wq_sb[:, dt_i] for dt_i in range(NDT)]

        kT_tiles = []
        qT_tiles = []

        def emit_qk_tile(which, w_tiles, ct):
            dst = p_qk.tile([P, NSEQ], BF16, tag=f"{which}T{ct}", name=f"{which}T{ct}")
            woff = ct * P
            for nch in range(NNC):
                acc = ps_mm.tile([P, 512], F32, tag="mm", name="acc")
                for dt_i in range(NDT):
                    nc.tensor.matmul(
                        acc,
                        w_tiles[dt_i][:, woff : woff + P],
                        xt_tiles[dt_i][:, nch * 512 : (nch + 1) * 512],
                        start=(dt_i == 0),
                        stop=(dt_i == NDT - 1),
                    )
                stage_copy(dst[:, nch * 512 : (nch + 1) * 512], acc)
            (kT_tiles if which == "k" else qT_tiles).append(dst)

        for ct in range(NPAIR):
            emit_qk_tile("k", wk_tiles, ct)
        emit_qk_tile("q", wq_tiles, 0)

        wout_sb = p_wout.tile([P, NPAIR, D], BF16, tag="wout")
        nc.gpsimd.dma_start(out=wout_sb, in_=wout.ap())
        wout_tiles = [wout_sb[:, ct] for ct in range(NPAIR)]

        # ---- attention: flat software pipeline across chunk boundaries ----
        # Per jp step of the CURRENT chunk: dots -> av of the step one behind
        # (possibly the previous chunk's tail) -> maybe one out-proj group ->
        # exp for this step. The PE's in-order queue therefore never waits on
        # an epilogue chain: epilogues and out-projections overlap the next
        # chunk's dots/av stream (>3.4us PE-idle gaps re-throttle HAM to
        # 1.2GHz, which is what made the naive ordering slow).
        aT_by_ic = {}  # ic -> [aT tile per hp]

        class Cctx:
            def __init__(self, ic, hp):
                self.ic, self.hp, self.i0 = ic, hp, ic * 512
                self.av_ps = [
                    ps_av.tile([65, 512], F32, tag="av", name=f"av{s}")
                    for s in range(2)
                ]
                self.aT = p_aT.tile([P, 512], BF16, tag="aT", name=f"aT{ic}_{hp}")
                aT_by_ic.setdefault(ic, []).append(self.aT)

        def emit_dots(c, jp):
            tiles = []
            for s in range(2):
                tiles.append(ps_mm.tile([P, 1024], F32, tag="mm", name="dots"))
            for half in range(2):
                for s in range(2):
                    r0 = s * DH
                    jtx = 2 * jp + half
                    nc.tensor.matmul(
                        tiles[s][:, half * 512 : (half + 1) * 512],
                        kT_tiles[c.hp][r0 : r0 + DH, jtx * P : (jtx + 1) * P],
                        qT_tiles[c.hp][r0 : r0 + DH, c.i0 : c.i0 + 512],
                        start=True,
                        stop=True,
                        tile_position=(r0, 0),
                    )
            return tiles

        def emit_exp(c, jp, dots_tiles):
            exp_tiles = []
            for s in range(2):
                e = p_exp.tile([P, 1024], BF16, tag="exp")
                if (jp, s) in DVE_EXP:
                    nc.vector.tensor_scalar(
                        out=e.bitcast(I16),
                        in0=dots_tiles[s],
                        scalar1=A_SCH,
                        scalar2=B_SCH,
                        op0=ALU.mult,
                        op1=ALU.add,
                    )
                else:
                    nc.scalar.activation(
                        out=e, in_=dots_tiles[s], func=AF.Exp, scale=SCALE
                    )
                exp_tiles.append(e)
            return exp_tiles

        def emit_av(c, jp, exp_pair):
            for s in range(2):
                sg = c.hp * 2 + s
                for half in range(2):
                    jtx = 2 * jp + half
                    nc.tensor.matmul(
                        c.av_ps[s],
                        v_tiles[jtx][:, sg * 65 : sg * 65 + 65],
                        exp_pair[s][:, half * 512 : (half + 1) * 512],
                        start=(jp == 0 and half == 0),
                        stop=(jp == NJP - 1 and half == 1),
                    )

        # Epilogue pipeline. The gpsimd must do NO compute here: its
        # partition_broadcast / tensor ops are ext-isa kernels that evict the
        # SWDGE DGE ucode from Q7 IRAM, and every swap costs a ~6us invisible
        # IRAM reload that was gating the PE once per chunk. Broadcast is a
        # K=1 ones-matmul on the PE instead; normalize-muls run on the DVE.
        def emit_epilogue_a(c, e):
            # evacuate av PSUM, hop denominator row to partition 0 (scalar
            # HWDGE queue: tiny, and ordered after the s=1 copy it needs)
            av_sb = []
            for s in range(2):
                t = p_avsb.tile([65, 512], F32, tag="av_sb", name=f"avsb{s}")
                if s == 0:
                    nc.vector.tensor_copy(t, c.av_ps[s])
                else:
                    nc.scalar.copy(t, c.av_ps[s])
                av_sb.append(t)
            e["av_sb"] = av_sb
            den_sb = p_den.tile([1, 1024], F32, tag="den_sb")
            for s in range(2):
                nc.scalar.dma_start(
                    out=den_sb[:, s * 512 : (s + 1) * 512], in_=av_sb[s][64:65, :]
                )
            e["den_sb"] = den_sb

        def emit_epilogue_b1(c, e):
            # two steps later: den hop has landed, recip doesn't block DVE.
            # SWDGE casting DMA converts f32 -> bf16 for the fp32r-averse
            # broadcast matmul (verifier rejects bitcast-f32r matmul inputs).
            recip = p_recip.tile([1, 1024], F32, tag="recip")
            nc.vector.reciprocal_approx_fast(out=recip, in_=e["den_sb"])
            recip_bf = p_recip.tile([1, 1024], BF16, tag="recip_bf")
            nc.gpsimd.dma_start(out=recip_bf, in_=recip)
            e["recip"] = recip_bf

        def emit_epilogue_b2(c, e):
            # PE broadcast: bc[d, i] = ones[d] * recip[i], K=1 matmul
            bcs = []
            for s in range(2):
                bc = ps_out.tile([DH, 512], F32, tag="o", name=f"bc{s}")
                nc.tensor.matmul(
                    bc,
                    ones_row,
                    e["recip"][:, s * 512 : (s + 1) * 512],
                    start=True,
                    stop=True,
                )
                bcs.append(bc)
            e["bc"] = bcs

        def emit_epilogue_b3(c, e):
            # normalize on DVE; s=1 reaches aT partitions 64:127 via an
            # SBUF-to-SBUF DMA hop (engines can't cross partitions)
            av_sb, bcs = e["av_sb"], e["bc"]
            nc.vector.tensor_mul(c.aT[0:DH, :], av_sb[0][0:DH, :], bcs[0])
            tmp = p_tmp.tile([DH, 512], BF16, tag="tmp")
            nc.vector.tensor_mul(tmp, av_sb[1][0:DH, :], bcs[1])
            nc.gpsimd.dma_start(out=c.aT[DH:P, :], in_=tmp)

        o_sb_by_nt = {}
        o_ps_cur = [None]

        def emit_outproj_mm(ic, lnt, ec, hp):
            # one accumulating matmul per pipeline step: each waits only on
            # its own hp's aT, so the PE never stalls on a full epilogue chain
            nt = 4 * ic + lnt
            if ec == 0 and hp == 0:
                o_sb_by_nt[nt] = p_ostage.tile(
                    [P, D], BF16, tag="o_sb", name=f"o_sb{nt}"
                )
            if hp == 0:
                o_ps_cur[0] = ps_out.tile([P, 512], F32, tag="o", name=f"o{nt}_{ec}")
            o_ps = o_ps_cur[0]
            nc.tensor.matmul(
                o_ps,
                aT_by_ic[ic][hp][:, lnt * P : (lnt + 1) * P],
                wout_tiles[hp][:, ec * 512 : (ec + 1) * 512],
                start=(hp == 0),
                stop=(hp == NPAIR - 1),
            )
            if hp == NPAIR - 1:
                stage_copy(o_sb_by_nt[nt][:, ec * 512 : (ec + 1) * 512], o_ps)
                if ec == 1:
                    nc.sync.dma_start(out=out_t[nt], in_=o_sb_by_nt.pop(nt))

        from collections import deque

        pend_av = deque()  # (cctx, jp, exp_tiles); av lags dots by AV_LAG steps
        AV_LAG = 3  # exp gets three pipeline steps of slack before av needs it
        pend_tasks = []  # [countdown, fn] epilogue stages
        pend_out = deque()  # (ic, lnt, ec, hp) mms
        out_gate = 0  # steps to wait before pumping out-proj (aT DMA latency)

        def pump(drain=False):
            nonlocal out_gate
            if pend_av and (len(pend_av) >= AV_LAG or drain):
                pc, pjp, pexp = pend_av.popleft()
                emit_av(pc, pjp, pexp)
                if pjp == NJP - 1:
                    e = {}
                    emit_epilogue_a(pc, e)
                    for delay, fn in (
                        (2, emit_epilogue_b1),
                        (4, emit_epilogue_b2),
                        (5, emit_epilogue_b3),
                    ):
                        pend_tasks.append([delay, fn, pc, e])
                    if pc.hp == NPAIR - 1 and pc.ic < NNC - 1:
                        pend_out.extend(
                            (pc.ic, lnt, ec, hp)
                            for lnt in range(4)
                            for ec in range(2)
                            for hp in range(NPAIR)
                        )
                        out_gate = 12
            for t in list(pend_tasks):
                t[0] -= 1
                if t[0] <= 0:
                    t[1](t[2], t[3])
                    pend_tasks.remove(t)
            if out_gate <= 0:
                for _ in range(2 if len(pend_out) > 8 else 1):
                    if pend_out:
                        emit_outproj_mm(*pend_out.popleft())
            out_gate -= 1

        chunks = [(ic, hp) for ic in range(NNC) for hp in range(NPAIR)]
        for ci, (ic, hp) in enumerate(chunks):
            c = Cctx(ic, hp)
            for jp in range(NJP):
                dots_tiles = emit_dots(c, jp)
                pump()
                exp_tiles = emit_exp(c, jp, dots_tiles)
                pend_av.append((c, jp, exp_tiles))
            if ic == 0 and hp + 1 < NPAIR:
                # remaining qT tiles ride between the first chunks' streams
                emit_qk_tile("q", wq_tiles, hp + 1)
        # drain av tail + epilogues
        while pend_av or pend_tasks or pend_out:
            pump(drain=True)

        # final-ic out-projection: the last chunk's epilogue chain (~6us)
        # would stall every group at its hp3 matmul with only 2 PSUM slots.
        # The dots/av banks are free now — borrow them so 6 groups run their
        # ready hp0-2 matmuls as cover while that epilogue completes.
        ICL = NNC - 1
        fgroups = [(lnt, ec) for lnt in range(4) for ec in range(2)]
        fpools = [
            (ps_out, "o"),
            (ps_out, "o"),
            (ps_av, "av"),
            (ps_av, "av"),
            (ps_mm, "mm"),
            (ps_mm, "mm"),
        ]
        o_ps_f = []
        for gi in range(6):
            lnt, ec = fgroups[gi]
            if ec == 0:
                o_sb_by_nt[4 * ICL + lnt] = p_ostage.tile(
                    [P, D], BF16, tag="o_sb", name=f"fsb{lnt}"
                )
            pool, ptag = fpools[gi]
            t = pool.tile([P, 512], F32, tag=ptag, name=f"fo{gi}")
            o_ps_f.append(t)
        for hp in range(NPAIR - 1):
            for gi in range(6):
                lnt, ec = fgroups[gi]
                nc.tensor.matmul(
                    o_ps_f[gi],
                    aT_by_ic[ICL][hp][:, lnt * P : (lnt + 1) * P],
                    wout_tiles[hp][:, ec * 512 : (ec + 1) * 512],
                    start=(hp == 0),
                    stop=False,
                )
        for gi in range(6):
            lnt, ec = fgroups[gi]
            nc.tensor.matmul(
                o_ps_f[gi],
                aT_by_ic[ICL][NPAIR - 1][:, lnt * P : (lnt + 1) * P],
                wout_tiles[NPAIR - 1][:, ec * 512 : (ec + 1) * 512],
                start=False,
                stop=True,
            )
            stage_copy(o_sb_by_nt[4 * ICL + lnt][:, ec * 512 : (ec + 1) * 512], o_ps_f[gi])
            if ec == 1:
                nc.sync.dma_start(
                    out=out_t[4 * ICL + lnt], in_=o_sb_by_nt.pop(4 * ICL + lnt)
                )
        for gi in (6, 7):
            lnt, ec = fgroups[gi]
            for hp in range(NPAIR):
                emit_outproj_mm(ICL, lnt, ec, hp)
        st_wq.close()
        st_wk.close()
        st_xt.close()

    nc.compile()
    return nc


_NC = None


def _get_program():
    global _NC
    if _NC is None:
        _NC = build_program()
    return _NC


INNER = 1024
BF = ml_dtypes.bfloat16


def kernel(x, W_qkv, W_out, b_out):
    x = np.asarray(x, dtype=np.float32)
    W_qkv = np.asarray(W_qkv, dtype=np.float32)
    W_out = np.asarray(W_out, dtype=np.float32)
    b_out = np.asarray(b_out, dtype=np.float32)
    B = x.shape[0]

    nc = _get_program()
    in_maps = []
    for b in range(B):
        for hh in range(2):
            cs = hh * CH
            wq = W_qkv[:, cs : cs + CH]
            wk = W_qkv[:, INNER + cs : INNER + cs + CH]
            wv = W_qkv[:, 2 * INNER + cs : 2 * INNER + cs + CH]
            # partition-major shuffles so per-partition DMA runs are contiguous
            xt_sh = np.ascontiguousarray(
                x[b].T.reshape(NDT, P, NSEQ).transpose(1, 0, 2)
            ).astype(BF)
            wqkv_sh = np.ascontiguousarray(
                np.stack([wq, wk, wv])  # [3, 1024, 512]
                .reshape(3, NDT, P, CH)
                .transpose(2, 0, 1, 3)
            ).astype(BF)
            wout_sh = np.ascontiguousarray(
                W_out[cs : cs + CH, :].reshape(NPAIR, P, D).transpose(1, 0, 2)
            ).astype(BF)
            in_maps.append(
                {
                    "xt": xt_sh,
                    "wqkv": wqkv_sh,
                    "wout": wout_sh,
                    "ones": np.ones((P, 1), dtype=BF),
                }
            )
    res = run_bass_kernel_spmd(nc, in_maps, core_ids=list(range(8)))
    out = np.empty((B, NSEQ, D), dtype=np.float32)
    for b in range(B):
        out[b] = (
            res.results[2 * b]["out"].astype(np.float32)
            + res.results[2 * b + 1]["out"].astype(np.float32)
            + b_out
        )
    return out


# revision 86
# speedup vs baseline: 1.0109x; 1.0059x over previous
"""Trainium2 Bass kernel for nn_Attention_86698209837214.

Multi-head attention: out = softmax(q k^T / 8) v @ W_out + b_out with
B=4, N=2048, DIM=1024, H=16, Dh=64.

Sharding: 8 cores = (batch b in 0..3) x (head-half hh in 0..1); each core
computes 8 heads of one batch. Host pre-transposes x[b], slices weights and
converts everything to bf16; host adds the two head-half partial outputs
plus b_out.

Device dataflow per core (bf16 operands, fp32 PSUM):
  1. v = x @ Wv in natural [n, c] layout, augmented with a ones column per
     head slot (row 64 of the attn@v accumulator = softmax denominator).
  2. qT, kT = (x @ Wq/Wk)^T in [c, n] layout (lhsT = W tiles).
  3. Attention, ic (i-chunk of 512) outer, hp (head pair) inner:
     dots^T per head via K=64 row-packed matmuls (tile_position r0=64*s, the
     two head streams run on disjoint PE row groups and overlap),
     exp split across two engines: ScalarE table exp (bf16 out) and DVE
     Schraudolph exp (tensor_scalar mult+add -> round-to-int16 == bf16 bit
     pattern; ~2% weight noise that cancels in softmax normalization),
     attn@v accumulated over j in PSUM with M=65 (65th row = denominator);
     attn@v lags dots by one jp so the PE never waits on the current exp.
     Epilogue: av -> SBUF copies (frees PSUM banks), denominator row to
     partition 0 via DMA hop, fast reciprocal, gpsimd partition_broadcast,
     gpsimd muls into aT (normalize); s=1 half reaches aT partitions 64:127
     via SBUF-to-SBUF DMA.
  4. Out-projection per ic accumulates all 4 head pairs in PSUM (K=128 x 4)
     -> single bf16 result DMA'd out; host adds the 2 cores + b_out.
"""

import sys

for _p in ("/opt/trn_rl_repo",):
    if _p not in sys.path:
        sys.path.append(_p)

from contextlib import ExitStack

import ml_dtypes
import numpy as np

import concourse.bass as bass  # noqa: F401
import concourse.tile as tile
from concourse import bacc, mybir
from concourse.bass_utils import run_bass_kernel_spmd

F32 = mybir.dt.float32
F32R = mybir.dt.float32r
BF16 = mybir.dt.bfloat16
I16 = mybir.dt.int16
AF = mybir.ActivationFunctionType
ALU = mybir.AluOpType

P = 128
NSEQ = 2048  # sequence length per batch
D = 1024  # model dim
CH = 512  # per-core head-dim width (8 heads x 64)
DH = 64
NPAIR = 4  # head pairs per core (c-tiles of 128)
NDT = D // P  # 8 d-tiles
NNT = NSEQ // P  # 16 n-tiles
NNC = NSEQ // 512  # 4 i-chunks
NJP = NNT // 2  # 8 jp steps per chunk
SCALE = 0.125  # DIM_HEAD ** -0.5

# Schraudolph exp in bf16-bit-pattern domain (scale folded in):
#   int16 bits = round(dots * SCALE * 2^7/ln2 + (127*128 - 486411/2^16))
A_SCH = SCALE * 184.6650390625
B_SCH = 16248.576

# (jp, s) pairs whose exp runs on DVE (Schraudolph); rest on ScalarE.
DVE_EXP = {(jp, 1) for jp in range(NSEQ // 256)}


def build_program():
    nc = bacc.Bacc("TRN2", target_bir_lowering=False, debug=False)

    # host-shuffled layouts: partition-major so each partition's DMA run is
    # long and contiguous (DMA engines are packet-rate-bound on short runs)
    xt = nc.dram_tensor("xt", [P, NDT, NSEQ], BF16, kind="ExternalInput")
    wqkv = nc.dram_tensor("wqkv", [P, 3, NDT, CH], BF16, kind="ExternalInput")
    wout = nc.dram_tensor("wout", [P, NPAIR, D], BF16, kind="ExternalInput")
    ones_in = nc.dram_tensor("ones", [P, 1], BF16, kind="ExternalInput")
    out = nc.dram_tensor("out", [NSEQ, D], BF16, kind="ExternalOutput")

    out_t = out.ap().rearrange("(nt p) e -> nt p e", p=P)  # [16, 128, 1024]

    copy_flip = [0]

    with tile.TileContext(nc) as tc, ExitStack() as ctx:
        # ---- persistent pools ----
        p_qk = ctx.enter_context(tc.tile_pool(name="p_qk", bufs=1))  # 32 KB/p
        p_v = ctx.enter_context(tc.tile_pool(name="p_v", bufs=1))  # ~16 KB/p
        p_small = ctx.enter_context(tc.tile_pool(name="p_small", bufs=1))
        # PSUM: dots 2x[128,1024] (4 banks) + av 2x[65,512] (2) + out 2x (2)
        ps_mm = ctx.enter_context(tc.tile_pool(name="ps_mm", bufs=2, space="PSUM"))
        ps_av = ctx.enter_context(tc.tile_pool(name="ps_av", bufs=2, space="PSUM"))
        ps_out = ctx.enter_context(tc.tile_pool(name="ps_out", bufs=2, space="PSUM"))
        # attention-phase pools (created before the transient phase-A pools so
        # pool release order stays LIFO)
        p_exp = ctx.enter_context(tc.tile_pool(name="p_exp", bufs=12))  # 24 KB/p
        p_aT = ctx.enter_context(tc.tile_pool(name="p_aT", bufs=16))  # 16 KB/p
        p_wout = ctx.enter_context(tc.tile_pool(name="p_wout", bufs=1))  # 8 KB/p
        p_avsb = ctx.enter_context(tc.tile_pool(name="p_avsb", bufs=3))  # 6 KB/p
        p_den = ctx.enter_context(tc.tile_pool(name="p_den", bufs=2))
        p_recip = ctx.enter_context(tc.tile_pool(name="p_recip", bufs=2))
        p_bcast = ctx.enter_context(tc.tile_pool(name="p_bcast", bufs=3))
        p_tmp = ctx.enter_context(tc.tile_pool(name="p_tmp", bufs=2))
        p_ostage = ctx.enter_context(tc.tile_pool(name="p_ostage", bufs=3))

        ones = p_small.tile([P, 1], BF16, tag="ones")
        nc.sync.dma_start(out=ones, in_=ones_in.ap())
        # dummy exp: pulls the ~2.7us ACT_TABLE_LOAD into the initial DMA wait
        warm = p_small.tile([P, 1], F32, tag="warm")
        nc.scalar.activation(out=warm, in_=ones, func=AF.Exp, scale=1.0)
        # ones row for the PE-side reciprocal broadcast (K=1 matmul)
        ones_row = p_small.tile([1, DH], BF16, tag="ones_row")
        nc.vector.memset(ones_row, 1.0)

        def stage_copy(dst, src):
            # alternate PSUM->SBUF staging copies between DVE and ScalarE
            copy_flip[0] ^= 1
            if copy_flip[0]:
                nc.vector.tensor_copy(dst, src)
            else:
                nc.scalar.copy(dst, src)

        # ---- phase A: load xt, wv, wk; compute v_aug ----
        st_xt = ExitStack()
        p_xt = st_xt.enter_context(tc.tile_pool(name="p_xt", bufs=1))  # 32 KB/p
        st_wk = ExitStack()
        p_wk = st_wk.enter_context(tc.tile_pool(name="p_wk", bufs=1))
        st_wv = ExitStack()
        p_wv = st_wv.enter_context(tc.tile_pool(name="p_wv", bufs=1))

        # input DMAs: per-dt slices of xt and wv round-robin across all three
        # DMA queues, so the first tiles land ~6us in and the dt-outer v-proj
        # below starts streaming long before the full load completes
        wv_sb = p_wv.tile([P, NDT, CH], BF16, tag="wv")
        xt_sb = p_xt.tile([P, NDT, NSEQ], BF16, tag="xt")
        dma_q = [nc.scalar, nc.sync, nc.gpsimd]
        # column-quartered AND half-pass-major: all first-half quarters for
        # every dt land before any second-half quarter, so the v-proj's first
        # pass (which needs q0/q1 of ALL dts) never queues behind data the
        # second pass won't touch until ~10us later
        for dt_i in range(NDT):
            q = dma_q[dt_i % 3]
            q.dma_start(out=wv_sb[:, dt_i], in_=wqkv.ap()[:, 2, dt_i])
            for cq in range(2):
                q.dma_start(
                    out=xt_sb[:, dt_i, cq * 512 : (cq + 1) * 512],
                    in_=xt.ap()[:, dt_i, cq * 512 : (cq + 1) * 512],
                )
        for dt_i in range(NDT):
            q = dma_q[dt_i % 3]
            for cq in range(2, 4):
                q.dma_start(
                    out=xt_sb[:, dt_i, cq * 512 : (cq + 1) * 512],
                    in_=xt.ap()[:, dt_i, cq * 512 : (cq + 1) * 512],
                )
        wk_sb = p_wk.tile([P, NDT, CH], BF16, tag="wk")
        nc.scalar.dma_start(out=wk_sb, in_=wqkv.ap()[:, 1])
        xt_tiles = [xt_sb[:, dt_i] for dt_i in range(NDT)]
        wv_tiles = [wv_sb[:, dt_i] for dt_i in range(NDT)]
        wk_tiles = [wk_sb[:, dt_i] for dt_i in range(NDT)]

        # v_aug: per head-slot sg, 65 cols = [v_sg (64) | ones (1)]
        v_tiles = []
        for nt in range(NNT):
            dst = p_v.tile([P, 8 * 65], BF16, tag=f"v{nt}")
            ones_dst = dst.rearrange("p (h c) -> p h c", c=65)[:, :, 64:65]
            nc.gpsimd.memset(ones_dst, 1.0)
            v_tiles.append(dst)
        # dt-outer over 8 parallel PSUM accumulators (all 8 banks are free at
        # startup): each dt's matmuls issue as soon as that dt slice arrives
        vacc = []
        for nm in ("vA", "vB"):
            t = ps_mm.tile([P, 1024], F32, tag="mm", name=nm)
            vacc += [t[:, 0:512], t[:, 512:1024]]
        for s in range(2):
            t = ps_av.tile([P, 512], F32, tag="av", name=f"vC{s}")
            vacc.append(t)
        for s in range(2):
            t = ps_out.tile([P, 512], F32, tag="o", name=f"vD{s}")
            vacc.append(t)
        for half in range(2):
            for dt_i in range(NDT):
                for k in range(8):
                    nt = half * 8 + k
                    nc.tensor.matmul(
                        vacc[k],
                        xt_tiles[dt_i][:, nt * P : (nt + 1) * P],
                        wv_tiles[dt_i],
                        start=(dt_i == 0),
                        stop=(dt_i == NDT - 1),
                    )
            for k in range(8):
                dst = v_tiles[half * 8 + k]
                v_dst = dst.rearrange("p (h c) -> p h c", c=65)[:, :, 0:DH]
                stage_copy(v_dst, vacc[k].rearrange("p (h c) -> p h c", c=DH))
        st_wv.close()

        # ---- phase B: kT c-tiles, then qT c-tiles interleaved with attention
        st_wq = ExitStack()
        p_wq = st_wq.enter_context(tc.tile_pool(name="p_wq", bufs=1))
        wq_sb = p_wq.tile([P, NDT, CH], BF16, tag="wq")
        nc.scalar.dma_start(out=wq_sb, in_=wqkv.ap()[:, 0])
        wq_tiles = [wq_sb[:, dt_i] for dt_i in range(NDT)]

        kT_tiles = []
        qT_tiles = []

        def emit_qk_tile(which, w_tiles, ct):
            dst = p_qk.tile([P, NSEQ], BF16, tag=f"{which}T{ct}", name=f"{which}T{ct}")
            woff = ct * P
            for nch in range(NNC):
                acc = ps_mm.tile([P, 512], F32, tag="mm", name="acc")
                for dt_i in range(NDT):
                    nc.tensor.matmul(
                        acc,
                        w_tiles[dt_i][:, woff : woff + P],
                        xt_tiles[dt_i][:, nch * 512 : (nch + 1) * 512],
                        start=(dt_i == 0),
                        stop=(dt_i == NDT - 1),
                    )
                stage_copy(dst[:, nch * 512 : (nch + 1) * 512], acc)
            (kT_tiles if which == "k" else qT_tiles).append(dst)

        for ct in range(NPAIR):
            emit_qk_tile("k", wk_tiles, ct)
        emit_qk_tile("q", wq_tiles, 0)

        wout_sb = p_wout.tile([P, NPAIR, D], BF16, tag="wout")
        nc.gpsimd.dma_start(out=wout_sb, in_=wout.ap())
        wout_tiles = [wout_sb[:, ct] for ct in range(NPAIR)]

        # ---- attention: flat software pipeline across chunk boundaries ----
        # Per jp step of the CURRENT chunk: dots -> av of the step one behind
        # (possibly the previous chunk's tail) -> maybe one out-proj group ->
        # exp for this step. The PE's in-order queue therefore never waits on
        # an epilogue chain: epilogues and out-projections overlap the next
        # chunk's dots/av stream (>3.4us PE-idle gaps re-throttle HAM to
        # 1.2GHz, which is what made the naive ordering slow).
        aT_by_ic = {}  # ic -> [aT tile per hp]

        class Cctx:
            def __init__(self, ic, hp):
                self.ic, self.hp, self.i0 = ic, hp, ic * 512
                self.av_ps = [
                    ps_av.tile([65, 512], F32, tag="av", name=f"av{s}")
                    for s in range(2)
                ]
                self.aT = p_aT.tile([P, 512], BF16, tag="aT", name=f"aT{ic}_{hp}")
                aT_by_ic.setdefault(ic, []).append(self.aT)

        def emit_dots(c, jp):
            tiles = []
            for s in range(2):
                tiles.append(ps_mm.tile([P, 1024], F32, tag="mm", name="dots"))
            for half in range(2):
                for s in range(2):
                    r0 = s * DH
                    jtx = 2 * jp + half
                    nc.tensor.matmul(
                        tiles[s][:, half * 512 : (half + 1) * 512],
                        kT_tiles[c.hp][r0 : r0 + DH, jtx * P : (jtx + 1) * P],
                        qT_tiles[c.hp][r0 : r0 + DH, c.i0 : c.i0 + 512],
                        start=True,
                        stop=True,
                        tile_position=(r0, 0),
                    )
            return tiles

        def emit_exp(c, jp, dots_tiles):
            exp_tiles = []
            for s in range(2):
                e = p_exp.tile([P, 1024], BF16, tag="exp")
                if (jp, s) in DVE_EXP:
                    nc.vector.tensor_scalar(
                        out=e.bitcast(I16),
                        in0=dots_tiles[s],
                        scalar1=A_SCH,
                        scalar2=B_SCH,
                        op0=ALU.mult,
                        op1=ALU.add,
                    )
                else:
                    nc.scalar.activation(
                        out=e, in_=dots_tiles[s], func=AF.Exp, scale=SCALE
                    )
                exp_tiles.append(e)
            return exp_tiles

        def emit_av(c, jp, exp_pair):
            for s in range(2):
                sg = c.hp * 2 + s
                for half in range(2):
                    jtx = 2 * jp + half
                    nc.tensor.matmul(
                        c.av_ps[s],
                        v_tiles[jtx][:, sg * 65 : sg * 65 + 65],
                        exp_pair[s][:, half * 512 : (half + 1) * 512],
                        start=(jp == 0 and half == 0),
                        stop=(jp == NJP - 1 and half == 1),
                    )

        # Epilogue pipeline. The gpsimd must do NO compute here: its
        # partition_broadcast / tensor ops are ext-isa kernels that evict the
        # SWDGE DGE ucode from Q7 IRAM, and every swap costs a ~6us invisible
        # IRAM reload that was gating the PE once per chunk. Broadcast is a
        # K=1 ones-matmul on the PE instead; normalize-muls run on the DVE.
        def emit_epilogue_a(c, e):
            # evacuate av PSUM, hop denominator row to partition 0 (scalar
            # HWDGE queue: tiny, and ordered after the s=1 copy it needs)
            av_sb = []
            for s in range(2):
                t = p_avsb.tile([65, 512], F32, tag="av_sb", name=f"avsb{s}")
                if s == 0:
                    nc.vector.tensor_copy(t, c.av_ps[s])
                else:
                    nc.scalar.copy(t, c.av_ps[s])
                av_sb.append(t)
            e["av_sb"] = av_sb
            den_sb = p_den.tile([1, 1024], F32, tag="den_sb")
            for s in range(2):
                nc.scalar.dma_start(
                    out=den_sb[:, s * 512 : (s + 1) * 512], in_=av_sb[s][64:65, :]
                )
            e["den_sb"] = den_sb

        def emit_epilogue_b1(c, e):
            # two steps later: den hop has landed, recip doesn't block DVE.
            # SWDGE casting DMA converts f32 -> bf16 for the fp32r-averse
            # broadcast matmul (verifier rejects bitcast-f32r matmul inputs).
            recip = p_recip.tile([1, 1024], F32, tag="recip")
            nc.vector.reciprocal_approx_fast(out=recip, in_=e["den_sb"])
            recip_bf = p_recip.tile([1, 1024], BF16, tag="recip_bf")
            nc.gpsimd.dma_start(out=recip_bf, in_=recip)
            e["recip"] = recip_bf

        def emit_epilogue_b2(c, e):
            # PE broadcast: bc[d, i] = ones[d] * recip[i], K=1 matmul
            bcs = []
            for s in range(2):
                bc = ps_out.tile([DH, 512], F32, tag="o", name=f"bc{s}")
                nc.tensor.matmul(
                    bc,
                    ones_row,
                    e["recip"][:, s * 512 : (s + 1) * 512],
                    start=True,
                    stop=True,
                )
                bcs.append(bc)
            e["bc"] = bcs

        def emit_epilogue_b3(c, e):
            # normalize on DVE; s=1 reaches aT partitions 64:127 via an
            # SBUF-to-SBUF DMA hop (engines can't cross partitions)
            av_sb, bcs = e["av_sb"], e["bc"]
            nc.vector.tensor_mul(c.aT[0:DH, :], av_sb[0][0:DH, :], bcs[0])
            tmp = p_tmp.tile([DH, 512], BF16, tag="tmp")
            nc.vector.tensor_mul(tmp, av_sb[1][0:DH, :], bcs[1])
            nc.gpsimd.dma_start(out=c.aT[DH:P, :], in_=tmp)

        o_sb_by_nt = {}
        o_ps_cur = [None]

        def emit_outproj_mm(ic, lnt, ec, hp):
            # one accumulating matmul per pipeline step: each waits only on
            # its own hp's aT, so the PE never stalls on a full epilogue chain
            nt = 4 * ic + lnt
            if ec == 0 and hp == 0:
                o_sb_by_nt[nt] = p_ostage.tile(
                    [P, D], BF16, tag="o_sb", name=f"o_sb{nt}"
                )
            if hp == 0:
                o_ps_cur[0] = ps_out.tile([P, 512], F32, tag="o", name=f"o{nt}_{ec}")
            o_ps = o_ps_cur[0]
            nc.tensor.matmul(
                o_ps,
                aT_by_ic[ic][hp][:, lnt * P : (lnt + 1) * P],
                wout_tiles[hp][:, ec * 512 : (ec + 1) * 512],
                start=(hp == 0),
                stop=(hp == NPAIR - 1),
            )
            if hp == NPAIR - 1:
                stage_copy(o_sb_by_nt[nt][:, ec * 512 : (ec + 1) * 512], o_ps)
                if ec == 1:
                    nc.sync.dma_start(out=out_t[nt], in_=o_sb_by_nt.pop(nt))

        from collections import deque

        pend_av = deque()  # (cctx, jp, exp_tiles); av lags dots by AV_LAG steps
        AV_LAG = 3  # exp gets three pipeline steps of slack before av needs it
        pend_tasks = []  # [countdown, fn] epilogue stages
        pend_out = deque()  # (ic, lnt, ec, hp) mms
        out_gate = 0  # steps to wait before pumping out-proj (aT DMA latency)

        def pump(drain=False):
            nonlocal out_gate
            if pend_av and (len(pend_av) >= AV_LAG or drain):
                pc, pjp, pexp = pend_av.popleft()
                emit_av(pc, pjp, pexp)
                if pjp == NJP - 1:
                    e = {}
                    emit_epilogue_a(pc, e)
                    for delay, fn in (
                        (2, emit_epilogue_b1),
                        (4, emit_epilogue_b2),
                        (5, emit_epilogue_b3),
                    ):
                        pend_tasks.append([delay, fn, pc, e])
                    if pc.hp == NPAIR - 1 and pc.ic < NNC - 1:
                        pend_out.extend(
                            (pc.ic, lnt, ec, hp)
                            for lnt in range(4)
                            for ec in range(2)
                            for hp in range(NPAIR)
                        )
                        out_gate = 12
            for t in list(pend_tasks):
                t[0] -= 1
                if t[0] <= 0:
                    t[1](t[2], t[3])
                    pend_tasks.remove(t)
            if out_gate <= 0:
                for _ in range(2 if len(pend_out) > 8 else 1):
                    if pend_out:
                        emit_outproj_mm(*pend_out.popleft())
            out_gate -= 1

        chunks = [(ic, hp) for ic in range(NNC) for hp in range(NPAIR)]
        for ci, (ic, hp) in enumerate(chunks):
            c = Cctx(ic, hp)
            for jp in range(NJP):
                dots_tiles = emit_dots(c, jp)
                pump()
                exp_tiles = emit_exp(c, jp, dots_tiles)
                pend_av.append((c, jp, exp_tiles))
            if ic == 0 and hp + 1 < NPAIR:
                # remaining qT tiles ride between the first chunks' streams
                emit_qk_tile("q", wq_tiles, hp + 1)
        # drain av tail + epilogues
        while pend_av or pend_tasks or pend_out:
            pump(drain=True)

        # final-ic out-projection: the last chunk's epilogue chain (~6us)
        # would stall every group at its hp3 matmul with only 2 PSUM slots.
        # The dots/av banks are free now — borrow them so 6 groups run their
        # ready hp0-2 matmuls as cover while that epilogue completes.
        ICL = NNC - 1
        fgroups = [(lnt, ec) for lnt in range(4) for ec in range(2)]
        fpools = [
            (ps_out, "o"),
            (ps_out, "o"),
            (ps_av, "av"),
            (ps_av, "av"),
            (ps_mm, "mm"),
            (ps_mm, "mm"),
        ]
        o_ps_f = []
        for gi in range(6):
            lnt, ec = fgroups[gi]
            if ec == 0:
                o_sb_by_nt[4 * ICL + lnt] = p_ostage.tile(
                    [P, D], BF16, tag="o_sb", name=f"fsb{lnt}"
                )
            pool, ptag = fpools[gi]
            t = pool.tile([P, 512], F32, tag=ptag, name=f"fo{gi}")
            o_ps_f.append(t)
        for hp in range(NPAIR - 1):
            for gi in range(6):
                lnt, ec = fgroups[gi]
                nc.tensor.matmul(
                    o_ps_f[gi],
                    aT_by_ic[ICL][hp][:, lnt * P : (lnt + 1) * P],
                    wout_tiles[hp][:, ec * 512 : (ec + 1) * 512],
                    start=(hp == 0),
                    stop=False,
                )
        for gi in range(6):
            lnt, ec = fgroups[gi]
            nc.tensor.matmul(
                o_ps_f[gi],
                aT_by_ic[ICL][NPAIR - 1][:, lnt * P : (lnt + 1) * P],
                wout_tiles[NPAIR - 1][:, ec * 512 : (ec + 1) * 512],
                start=False,
                stop=True,
            )
            stage_copy(o_sb_by_nt[4 * ICL + lnt][:, ec * 512 : (ec + 1) * 512], o_ps_f[gi])
            if ec == 1:
                nc.sync.dma_start(
                    out=out_t[4 * ICL + lnt], in_=o_sb_by_nt.pop(4 * ICL + lnt)
                )
        for gi in (6, 7):
            lnt, ec = fgroups[gi]
            for hp in range(NPAIR):
                emit_outproj_mm(ICL, lnt, ec, hp)
        st_wq.close()
        st_wk.close()
        st_xt.close()

    nc.compile()
    return nc


_NC = None


def _get_program():
    global _NC
    if _NC is None:
        _NC = build_program()
    return _NC


INNER = 1024
BF = ml_dtypes.bfloat16


def kernel(x, W_qkv, W_out, b_out):
    x = np.asarray(x, dtype=np.float32)
    W_qkv = np.asarray(W_qkv, dtype=np.float32)
    W_out = np.asarray(W_out, dtype=np.float32)
    b_out = np.asarray(b_out, dtype=np.float32)
    B = x.shape[0]

    nc = _get_program()
    in_maps = []
    for b in range(B):
        for hh in range(2):
            cs = hh * CH
            wq = W_qkv[:, cs : cs + CH]
            wk = W_qkv[:, INNER + cs : INNER + cs + CH]
            wv = W_qkv[:, 2 * INNER + cs : 2 * INNER + cs + CH]
            # partition-major shuffles so per-partition DMA runs are contiguous
            xt_sh = np.ascontiguousarray(
                x[b].T.reshape(NDT, P, NSEQ).transpose(1, 0, 2)
            ).astype(BF)
            wqkv_sh = np.ascontiguousarray(
                np.stack([wq, wk, wv])  # [3, 1024, 512]
                .reshape(3, NDT, P, CH)
                .transpose(2, 0, 1, 3)
            ).astype(BF)
            wout_sh = np.ascontiguousarray(
                W_out[cs : cs + CH, :].reshape(NPAIR, P, D).transpose(1, 0, 2)
            ).astype(BF)
            in_maps.append(
                {
                    "xt": xt_sh,
                    "wqkv": wqkv_sh,
                    "wout": wout_sh,
                    "ones": np.ones((P, 1), dtype=BF),
                }
            )
    res = run_bass_kernel_spmd(nc, in_maps, core_ids=list(range(8)))
    out = np.empty((B, NSEQ, D), dtype=np.float32)
    for b in range(B):
        out[b] = (
            res.results[2 * b]["out"].astype(np.float32)
            + res.results[2 * b + 1]["out"].astype(np.float32)
            + b_out
        )
    return out
